# Initial kernel scaffold
#
"""Trainium2 Bass kernel: 3D-RoPE multi-head attention (B=4,N=2048,DIM=1536,H=16,DH=96).

Sharding: 8 cores = (batch b = c//2) x (query half qh = c%2). Each core:
  - projects K,V for all 2048 tokens of its batch, Q for its 1024 query rows
  - applies RoPE to Q,K (cos/sin computed on device from raw freqs)
  - attention (softmax over keys, no max-subtraction: |score*scale| is O(5))
  - output projection for its 1024 rows -> full output rows, host gather = concat
All matmuls run as float32r (full-rate fp32 path, fp32 accumulation in PSUM).
"""

import sys

if "/opt/trn_rl_repo" not in sys.path:
    sys.path.insert(0, "/opt/trn_rl_repo")

import numpy as np

import concourse.bass as bass
import concourse.mybir as mybir
import concourse.tile as tile
from concourse import bacc
from concourse.bass_utils import run_bass_kernel_spmd

B, N, DIM, H, DH = 4, 2048, 1536, 16, 96
NQ = N // 2  # queries per core
SCALE = DH ** -0.5
KT = DIM // 128  # 12 contraction tiles
TT = 512  # moving-dim token tile
F32 = mybir.dt.float32
F32R = mybir.dt.float32r
BF16 = mybir.dt.bfloat16
IN_DT = BF16  # dtype of all matmul-input tiles/params (F32R | bfloat16)
import ml_dtypes
IN_NP = ml_dtypes.bfloat16
AF = mybir.ActivationFunctionType
HALF_PI = float(np.pi / 2)


def _r(ap):
    return ap


def _build_rope_pt() -> np.ndarray:
    """lhsT for the rotate_half permutation: rot(t) = PT.T @ t.

    Per 32-chunk c (3 chunks), within-chunk index a:
      a <  16: rot[base+a] = -t[base+a+16]
      a >= 16: rot[base+a] = +t[base+a-16]
    """
    P = np.zeros((DH, DH), np.float32)
    for c in range(3):
        base = 32 * c
        for a in range(16):
            P[base + a, base + a + 16] = -1.0
            P[base + 16 + a, base + a] = 1.0
    return np.ascontiguousarray(P.T)  # PT[j, i] = P[i, j]


def _emit(ctx, tc, io):
    nc = tc.nc
    xqT, xbT, fqT, fkT, ropePT, Wqkv, Wout, bout, out = io

    persist = ctx.enter_context(tc.tile_pool(name="persist", bufs=1))
    dram = ctx.enter_context(tc.tile_pool(name="dram", bufs=1, space="DRAM"))

    # ---- constants ------------------------------------------------------
    ropeP_sb = persist.tile([DH, DH], IN_DT, tag="ropeP")
    nc.sync.dma_start(out=ropeP_sb, in_=ropePT)
    ones1f = persist.tile([1, DH], F32, tag="ones1f")
    nc.vector.memset(ones1f, 1.0)
    ones1 = persist.tile([1, DH], F32R, tag="ones1")
    nc.scalar.copy(out=ones1, in_=ones1f)
    onescol = persist.tile([128, N // 128, 1], F32, tag="onescol")
    nc.vector.memset(onescol, 1.0)
    halfpi = persist.tile([DH, 1], F32, tag="halfpi")
    nc.vector.memset(halfpi, HALF_PI)

    KTspill = dram.tile([H, DH, N], IN_DT, tag="ktspill")
    Vspill = dram.tile([N, H * DH], IN_DT, tag="vspill")
    Wqkv_r = Wqkv.rearrange("(t p) c -> p t c", p=128)  # [128, KT, 4608]

    # cos/sin for queries (kept on the main stack; small)
    pqtrig = ctx.enter_context(tc.tile_pool(name="pqtrig", bufs=1))
    cosq = pqtrig.tile([DH, NQ], F32, tag="cosq")
    sinq = pqtrig.tile([DH, NQ], F32, tag="sinq")

    def rope(ps, cos_sl, sin_sl, dest, pool, psum_rot, tt=TT):
        """dest = cos*t + sin*(P @ t), t = ps (PSUM [DH, tt])."""
        t_sb = pool.tile([DH, TT], IN_DT, tag="rope_t", name="rope_t")
        nc.scalar.copy(out=t_sb[:, :tt], in_=ps)
        rot_ps = psum_rot.tile([DH, TT], F32, tag="rope_rot", name="rope_rot")
        nc.tensor.matmul(
            out=rot_ps[:, :tt], lhsT=_r(ropeP_sb), rhs=_r(t_sb[:, :tt]),
            start=True, stop=True,
        )
        u = pool.tile([DH, TT], F32, tag="rope_u", name="rope_u")
        nc.vector.tensor_mul(out=u[:, :tt], in0=t_sb[:, :tt], in1=cos_sl)
        w = pool.tile([DH, TT], F32, tag="rope_w", name="rope_w")
        nc.vector.tensor_mul(out=w[:, :tt], in0=rot_ps[:, :tt], in1=sin_sl)
        nc.vector.tensor_add(out=dest, in0=u[:, :tt], in1=w[:, :tt])

    # ---- Phase A+C+D1: trig, V projection, K projection (xbT resident) --
    with (
        tc.tile_pool(name="cd", bufs=1) as pcd,
        tc.tile_pool(name="ptrig", bufs=1) as ptrig,
    ):
        fq_sb = ptrig.tile([DH, NQ], F32, tag="fq")
        nc.sync.dma_start(out=fq_sb, in_=fqT)
        fk_sb = ptrig.tile([DH, N], F32, tag="fk")
        nc.sync.dma_start(out=fk_sb, in_=fkT)
        cosk = pcd.tile([DH, N], F32, tag="cosk")
        sink = pcd.tile([DH, N], F32, tag="sink")
        nc.scalar.activation(out=sinq, in_=fq_sb, func=AF.Sin)
        nc.scalar.activation(out=cosq, in_=fq_sb, func=AF.Sin, bias=halfpi)
        nc.scalar.activation(out=sink, in_=fk_sb, func=AF.Sin)
        nc.scalar.activation(out=cosk, in_=fk_sb, func=AF.Sin, bias=halfpi)

        xb_sb = [pcd.tile([128, N], IN_DT, tag=f"xb{k}", name=f"xb{k}")
                 for k in range(KT)]
        for k in range(KT):
            nc.sync.dma_start(out=xb_sb[k], in_=xbT[k * 128:(k + 1) * 128, :])

        # V: natural layout [token, vcol], 384-wide col tiles (4 heads)
        with (
            tc.tile_pool(name="wv", bufs=16) as pwv,
            tc.tile_pool(name="vsb", bufs=4) as pvsb,
            tc.tile_pool(name="psv", bufs=4, space="PSUM") as psv,
        ):
            for vt in range(4):
                cs = 2 * H * DH + vt * 384
                wv = [pwv.tile([128, 384], IN_DT, tag="wv", name=f"wv{vt}_{k}")
                      for k in range(KT)]
                for k in range(KT):
                    nc.sync.dma_start(out=wv[k], in_=Wqkv_r[:, k, cs:cs + 384])
                for tt_i in range(N // 128):
                    ps = psv.tile([128, 384], F32, tag="vps", name="vps")
                    for k in range(KT):
                        nc.tensor.matmul(
                            out=ps,
                            lhsT=_r(xb_sb[k][:, tt_i * 128:(tt_i + 1) * 128]),
                            rhs=_r(wv[k]),
                            start=(k == 0), stop=(k == KT - 1),
                        )
                    v_sb = pvsb.tile([128, 384], IN_DT, tag="v_sb", name="v_sb")
                    nc.vector.tensor_copy(out=v_sb, in_=ps)
                    nc.sync.dma_start(
                        out=Vspill[tt_i * 128:(tt_i + 1) * 128,
                                   vt * 384:(vt + 1) * 384],
                        in_=v_sb,
                    )

        # K: transposed layout per head [DH, token] + RoPE, spilled to DRAM
        with (
            tc.tile_pool(name="wk", bufs=16) as pwk,
            tc.tile_pool(name="ropek", bufs=3) as prk,
            tc.tile_pool(name="ktsp", bufs=3) as pkt,
            tc.tile_pool(name="psk", bufs=4, space="PSUM") as psk,
            tc.tile_pool(name="pskr", bufs=2, space="PSUM") as pskr,
        ):
            for h in range(H):
                cs = H * DH + h * DH
                wk = [pwk.tile([128, DH], IN_DT, tag="wk", name=f"wk{h}_{k}")
                      for k in range(KT)]
                for k in range(KT):
                    nc.sync.dma_start(out=wk[k], in_=Wqkv_r[:, k, cs:cs + DH])
                for kt_i in range(N // TT):
                    ps = psk.tile([DH, TT], F32, tag="kps", name="kps")
                    for k in range(KT):
                        nc.tensor.matmul(
                            out=ps, lhsT=_r(wk[k]),
                            rhs=_r(xb_sb[k][:, kt_i * TT:(kt_i + 1) * TT]),
                            start=(k == 0), stop=(k == KT - 1),
                        )
                    sl = slice(kt_i * TT, (kt_i + 1) * TT)
                    kt_sb = pkt.tile([DH, TT], IN_DT, tag="ktsp", name="ktsp")
                    rope(ps, cosk[:, sl], sink[:, sl], kt_sb[:, :], prk, pskr)
                    nc.sync.dma_start(out=KTspill[h, :, sl], in_=kt_sb)

    # ---- Phase B: Q projection + RoPE (uses xqT) -------------------------
    pqt = ctx.enter_context(tc.tile_pool(name="qtp", bufs=1))
    QTs = [pqt.tile([DH, NQ], IN_DT, tag=f"qt{h}", name=f"qt{h}")
           for h in range(H)]
    with (
        tc.tile_pool(name="xq", bufs=1) as pxq,
        tc.tile_pool(name="wq", bufs=16) as pwq,
        tc.tile_pool(name="ropeb", bufs=3) as prb,
        tc.tile_pool(name="psb", bufs=4, space="PSUM") as psb,
        tc.tile_pool(name="psbr", bufs=2, space="PSUM") as psbr,
    ):
        xq_sb = [pxq.tile([128, NQ], IN_DT, tag=f"xq{k}", name=f"xq{k}")
                 for k in range(KT)]
        for k in range(KT):
            nc.sync.dma_start(out=xq_sb[k], in_=xqT[k * 128:(k + 1) * 128, :])
        for h in range(H):
            wq = [pwq.tile([128, DH], IN_DT, tag="wq", name=f"wq{h}_{k}")
                  for k in range(KT)]
            for k in range(KT):
                nc.sync.dma_start(out=wq[k], in_=Wqkv_r[:, k, h * DH:(h + 1) * DH])
            for qt in range(NQ // TT):
                ps = psb.tile([DH, TT], F32, tag="qps", name="qps")
                for k in range(KT):
                    nc.tensor.matmul(
                        out=ps, lhsT=_r(wq[k]),
                        rhs=_r(xq_sb[k][:, qt * TT:(qt + 1) * TT]),
                        start=(k == 0), stop=(k == KT - 1),
                    )
                sl = slice(qt * TT, (qt + 1) * TT)
                rope(ps, cosq[:, sl], sinq[:, sl], QTs[h][:, sl], prb, psbr)
    # ---- Phase D2: attention per head ------------------------------------
    pho = ctx.enter_context(tc.tile_pool(name="hop", bufs=1))
    hoTs = [pho.tile([DH, NQ], IN_DT, tag=f"ho{h}", name=f"ho{h}")
            for h in range(H)]
    with (
        tc.tile_pool(name="kth", bufs=2) as pkth,
        tc.tile_pool(name="v1", bufs=2) as pv1,
        tc.tile_pool(name="ex", bufs=4) as pex,
        tc.tile_pool(name="small", bufs=4) as psm,
        tc.tile_pool(name="psho", bufs=3, space="PSUM") as psho,
        tc.tile_pool(name="pssc", bufs=2, space="PSUM") as pssc,
        tc.tile_pool(name="psbc", bufs=1, space="PSUM") as psbc,
    ):
        V_r = Vspill.rearrange("(mt p) c -> p mt c", p=128)
        NMT = N // 128
        for h in range(H):
            kt_h = pkth.tile([DH, N], IN_DT, tag="kth", name="kth")
            nc.sync.dma_start(out=kt_h, in_=KTspill[h])
            v1 = pv1.tile([128, NMT, DH + 1], IN_DT, tag="v1", name="v1")
            nc.sync.dma_start(
                out=v1[:, :, 0:DH], in_=V_r[:, :, h * DH:(h + 1) * DH]
            )
            nc.scalar.copy(out=v1[:, :, DH:DH + 1], in_=onescol)
            for qt in range(NQ // TT):
                ho_ps = psho.tile([DH + 1, TT], F32, tag="hops", name="hops")
                qsl = slice(qt * TT, (qt + 1) * TT)
                for mt2 in range(NMT // 2):
                    sc_ps = pssc.tile([128, 2 * TT], F32, tag="scps", name="scps")
                    for j in range(2):
                        mt = 2 * mt2 + j
                        nc.tensor.matmul(
                            out=sc_ps[:, j * TT:(j + 1) * TT],
                            lhsT=_r(kt_h[:, mt * 128:(mt + 1) * 128]),
                            rhs=_r(QTs[h][:, qsl]), start=True, stop=True,
                        )
                    ex = pex.tile([128, 2 * TT], IN_DT, tag="ex", name="ex")
                    nc.scalar.activation(out=ex, in_=sc_ps, func=AF.Exp,
                                         scale=SCALE)
                    for j in range(2):
                        mt = 2 * mt2 + j
                        nc.tensor.matmul(
                            out=ho_ps, lhsT=_r(v1[:, mt, :]),
                            rhs=_r(ex[:, j * TT:(j + 1) * TT]),
                            start=(mt == 0), stop=(mt == NMT - 1),
                        )
                # normalize: hoT = ho[0:DH] * bcast(1/denom), denom = row DH
                hoU = psm.tile([DH + 1, TT], IN_DT, tag="hoU", name="hoU")
                nc.scalar.copy(out=hoU[0:DH, :], in_=ho_ps[0:DH, :])
                rcp96 = psm.tile([DH + 1, TT], F32, tag="rcp96", name="rcp96")
                nc.vector.reciprocal(
                    out=rcp96[DH:DH + 1, :], in_=ho_ps[DH:DH + 1, :]
                )
                rcpf = psm.tile([1, TT], F32, tag="rcpf", name="rcpf")
                nc.sync.dma_start(out=rcpf, in_=rcp96[DH:DH + 1, :])
                rcp = psm.tile([1, TT], F32R, tag="rcp", name="rcp")
                nc.scalar.copy(out=rcp, in_=rcpf)
                bc_ps = psbc.tile([DH, TT], F32, tag="bcps", name="bcps")
                nc.tensor.matmul(
                    out=bc_ps, lhsT=_r(ones1), rhs=_r(rcp), start=True,
                    stop=True,
                )
                nc.vector.tensor_mul(
                    out=hoTs[h][:, qsl], in0=hoU[0:DH, :], in1=bc_ps
                )
    # ---- Phase E: output projection -------------------------------------
    with (
        tc.tile_pool(name="obias", bufs=1) as pob,
        tc.tile_pool(name="osb", bufs=4) as posb,
        tc.tile_pool(name="pse", bufs=4, space="PSUM") as pse,
    ):
        bias_sb = pob.tile([128, DIM], F32, tag="bias")
        bout_bc = bass.AP(tensor=bout.tensor, offset=bout.offset,
                          ap=[[0, 128]] + [list(p) for p in bout.ap])
        nc.sync.dma_start(out=bias_sb, in_=bout_bc)
        for et in range(DIM // TT):
            # reuse the (now dead) QT slots for the Wout tiles
            wo = [pqt.tile([DH, TT], IN_DT, tag=f"qt{h}", name=f"wo{et}_{h}")
                  for h in range(H)]
            for h in range(H):
                nc.sync.dma_start(
                    out=wo[h],
                    in_=Wout[h * DH:(h + 1) * DH, et * TT:(et + 1) * TT],
                )
            for tt_i in range(NQ // 128):
                ps = pse.tile([128, TT], F32, tag="eps", name="eps")
                for h in range(H):
                    nc.tensor.matmul(
                        out=ps,
                        lhsT=_r(hoTs[h][:, tt_i * 128:(tt_i + 1) * 128]),
                        rhs=_r(wo[h]),
                        start=(h == 0), stop=(h == H - 1),
                    )
                osb = posb.tile([128, TT], F32, tag="osb", name="osb")
                nc.vector.tensor_add(
                    out=osb, in0=ps, in1=bias_sb[:, et * TT:(et + 1) * TT]
                )
                nc.sync.dma_start(
                    out=out[tt_i * 128:(tt_i + 1) * 128, et * TT:(et + 1) * TT],
                    in_=osb,
                )


def build():
    from contextlib import ExitStack

    nc = bacc.Bacc("TRN2", target_bir_lowering=False, debug=False)
    xqT = nc.declare_dram_parameter("xqT", [DIM, NQ], IN_DT, isOutput=False)
    xbT = nc.declare_dram_parameter("xbT", [DIM, N], IN_DT, isOutput=False)
    fqT = nc.declare_dram_parameter("fqT", [DH, NQ], F32, isOutput=False)
    fkT = nc.declare_dram_parameter("fkT", [DH, N], F32, isOutput=False)
    ropePT = nc.declare_dram_parameter("ropePT", [DH, DH], IN_DT, isOutput=False)
    Wqkv_p = nc.declare_dram_parameter("Wqkv", [DIM, 3 * H * DH], IN_DT, isOutput=False)
    Wout_p = nc.declare_dram_parameter("Wout", [H * DH, DIM], IN_DT, isOutput=False)
    bout_p = nc.declare_dram_parameter("bout", [DIM], F32, isOutput=False)
    out = nc.declare_dram_parameter("out", [NQ, DIM], F32, isOutput=True)
    io = tuple(
        t[:] for t in (xqT, xbT, fqT, fkT, ropePT, Wqkv_p, Wout_p, bout_p, out)
    )
    with ExitStack() as ctx:
        tc = ctx.enter_context(tile.TileContext(nc))
        _emit(ctx, tc, io)
    nc.finalize()
    return nc


def make_in_maps(x, f1, f2, f3, Wqkv, Wout, bout):
    x = np.ascontiguousarray(np.asarray(x, np.float32))
    fcat = np.concatenate(
        [np.asarray(f1, np.float32), np.asarray(f2, np.float32),
         np.asarray(f3, np.float32)], axis=1,
    )  # [N, DH]
    fkT_np = np.ascontiguousarray(fcat.T)
    PT = _build_rope_pt()
    Wqkv = np.ascontiguousarray(np.asarray(Wqkv, np.float32))
    Wout = np.ascontiguousarray(np.asarray(Wout, np.float32))
    bout = np.ascontiguousarray(np.asarray(bout, np.float32))
    in_maps = []
    for c in range(8):
        b, qh = divmod(c, 2)
        qs = qh * NQ
        in_maps.append(dict(
            xqT=np.ascontiguousarray(x[b, qs:qs + NQ].T).astype(IN_NP),
            xbT=np.ascontiguousarray(x[b].T).astype(IN_NP),
            fqT=np.ascontiguousarray(fcat[qs:qs + NQ].T),
            fkT=fkT_np, ropePT=PT.astype(IN_NP), Wqkv=Wqkv.astype(IN_NP),
            Wout=Wout.astype(IN_NP), bout=bout,
        ))
    return in_maps


_NC_CACHE = None


def kernel(x, f1, f2, f3, Wqkv, Wout, bout, _trace=False):
    global _NC_CACHE
    if _NC_CACHE is None:
        _NC_CACHE = build()
    nc = _NC_CACHE
    in_maps = make_in_maps(x, f1, f2, f3, Wqkv, Wout, bout)
    res = run_bass_kernel_spmd(nc, in_maps, list(range(8)), trace=_trace)
    out = np.empty((B, N, DIM), np.float32)
    for c in range(8):
        b, qh = divmod(c, 2)
        out[b, qh * NQ:(qh + 1) * NQ] = res.results[c]["out"]
    if _trace:
        return out, res
    return out



# revision 28
# speedup vs baseline: 1.6556x; 1.6556x over previous
"""Trainium2 Bass kernel: 3D-RoPE multi-head attention (B=4,N=2048,DIM=1536,H=16,DH=96).

Sharding: 8 cores = (batch b = c//2) x (head half hh = c%2, 8 heads each).
Each core computes, for its batch and its 8 heads:
  - K,V,Q projections for all 2048 tokens (K/V/Q fully SBUF-resident)
  - RoPE on Q,K (cos/sin computed on device from raw freqs)
  - attention (softmax over keys via appended-ones column, no max-subtraction)
  - partial output projection (row-split Wout) -> host sums the two partials
    per batch. Bias is fed to the hh==0 core only (hh==1 gets zeros).
All matmul inputs are bf16; accumulation is fp32 in PSUM.
"""

import sys

if "/opt/trn_rl_repo" not in sys.path:
    sys.path.insert(0, "/opt/trn_rl_repo")

import numpy as np

import concourse.bass as bass
import concourse.mybir as mybir
import concourse.tile as tile
from concourse import bacc
from concourse.bass_utils import run_bass_kernel_spmd

B, N, DIM, H, DH = 4, 2048, 1536, 16, 96
HC = H // 2          # heads per core
HD = HC * DH         # 768 projected cols per core
SCALE = DH ** -0.5
KT = DIM // 128      # 12 contraction tiles
TT = 512             # query tile
NMT = N // 128       # 16 key chunks
NQT = N // TT        # 4 query tiles
F32 = mybir.dt.float32
F32R = mybir.dt.float32r
BF16 = mybir.dt.bfloat16
IN_DT = BF16
import ml_dtypes
IN_NP = ml_dtypes.bfloat16
AF = mybir.ActivationFunctionType
HALF_PI = float(np.pi / 2)


def _build_rope_pt() -> np.ndarray:
    """lhsT for the rotate_half permutation: rot(t) = PT.T @ t."""
    P = np.zeros((DH, DH), np.float32)
    for c in range(3):
        base = 32 * c
        for a in range(16):
            P[base + a, base + a + 16] = -1.0
            P[base + 16 + a, base + a] = 1.0
    return np.ascontiguousarray(P.T)


def _emit(ctx, tc, io):
    nc = tc.nc
    xT, fN, ident, Wq, Wk, Wv, WoC, boutC, out = io

    persist = ctx.enter_context(tc.tile_pool(name="persist", bufs=1))

    # ---- constants ------------------------------------------------------
    ones1f = persist.tile([1, DH], F32, tag="ones1f")
    nc.vector.memset(ones1f, 1.0)
    ones1 = persist.tile([1, DH], F32R, tag="ones1")
    nc.scalar.copy(out=ones1, in_=ones1f)
    halfpi = persist.tile([128, 1], F32, tag="halfpi")
    nc.vector.memset(halfpi, HALF_PI)

    ident_sb = persist.tile([128, 128], IN_DT, tag="ident")
    cosT = persist.tile([128, NMT, DH], IN_DT, tag="cosT")
    sinT = persist.tile([128, NMT, DH], IN_DT, tag="sinT")
    nsinT = persist.tile([128, NMT, DH], IN_DT, tag="nsinT")
    KTs = [persist.tile([DH, N], IN_DT, tag=f"kt{h}", name=f"kt{h}")
           for h in range(HC)]
    QTs = [persist.tile([DH, N], IN_DT, tag=f"qt{h}", name=f"qt{h}")
           for h in range(HC)]
    Vt = persist.tile([128, NMT, HC, DH + 1], IN_DT, tag="vt")
    nc.vector.memset(Vt[:, :, :, DH:DH + 1], 1.0)

    # ---- merged QKV projection in token layout --------------------------
    # One pass over 16 token tiles; per tile 6 column chunks of 384
    # (Q heads 0-3, Q 4-7, K 0-3, K 4-7, V 0-3, V 4-7), all with the full
    # 128-wide contraction. Q/K get RoPE via free-dim strided ops, then a
    # PE transpose into the [dh, token] layout attention wants.
    Wq_r = Wq.rearrange("(t p) c -> p t c", p=128)
    Wk_r = Wk.rearrange("(t p) c -> p t c", p=128)
    Wv_r = Wv.rearrange("(t p) c -> p t c", p=128)
    xT_r = xT.rearrange("(k p) n -> p k n", p=128)
    fN_r = fN.rearrange("(t p) c -> p t c", p=128)

    def bc_heads(base):
        """broadcast a [128, ...] AP across 4 heads via a 0-stride dim."""
        return bass.AP(tensor=base.tensor, offset=base.offset,
                       ap=[list(base.ap[0]), [0, 4]]
                          + [list(p) for p in base.ap[1:]])

    with (
        tc.tile_pool(name="pw", bufs=1) as pw,
        tc.tile_pool(name="pxt", bufs=3) as pxt,
        tc.tile_pool(name="ptrg", bufs=1) as ptrg,
        tc.tile_pool(name="prp", bufs=2) as prp,
        tc.tile_pool(name="psp", bufs=3, space="PSUM") as psp,
        tc.tile_pool(name="pstp", bufs=3, space="PSUM") as pstp,
    ):
        W_sb = pw.tile([128, KT, 3 * HD], IN_DT, tag="w")
        fN_sb = ptrg.tile([128, NMT, DH], F32, tag="fN")
        nc.sync.dma_start(out=fN_sb, in_=fN_r)
        nc.sync.dma_start(out=ident_sb, in_=ident)
        nc.sync.dma_start(out=W_sb[:, :, 0:384], in_=Wq_r[:, :, 0:384])
        nc.scalar.activation(out=sinT, in_=fN_sb, func=AF.Sin)
        nc.scalar.activation(out=cosT, in_=fN_sb, func=AF.Sin, bias=halfpi)
        nc.scalar.activation(out=nsinT, in_=fN_sb, func=AF.Sin, scale=-1.0)

        def xt_dma(t):
            xt = pxt.tile([128, KT, 128], IN_DT, tag="xt", name=f"xt{t}")
            nc.sync.dma_start(out=xt, in_=xT_r[:, :, t * 128:(t + 1) * 128])
            return xt
        xt_cur = xt_dma(0)
        nc.sync.dma_start(out=W_sb[:, :, 384:HD], in_=Wq_r[:, :, 384:HD])
        nc.sync.dma_start(out=W_sb[:, :, HD:2 * HD], in_=Wk_r)
        nc.sync.dma_start(out=W_sb[:, :, 2 * HD:3 * HD], in_=Wv_r)

        def half16(tile, t, half):
            """[128, 3, 16] view of one rotate-half of a [128, DH] slice."""
            base = tile[:, t, :]
            return bass.AP(tensor=base.tensor, offset=base.offset + 16 * half,
                           ap=[list(base.ap[0]), [32, 3], [1, 16]])

        for t in range(NMT):
            xt = xt_cur
            if t + 1 < NMT:
                xt_cur = xt_dma(t + 1)
            for c in range(6):
                pp = psp.tile([128, 4, 3, 32], F32, tag="pp", name="pp")
                for k in range(KT):
                    nc.tensor.matmul(
                        out=pp, lhsT=xt[:, k, :],
                        rhs=W_sb[:, k, c * 384:(c + 1) * 384],
                        start=(k == 0), stop=(k == KT - 1),
                    )
                if c >= 4:
                    j = c - 4
                    nc.scalar.copy(out=Vt[:, t, 4 * j:4 * j + 4, 0:DH],
                                   in_=pp)
                    continue
                # RoPE: dest = pp*cos + rot(pp)*sin; the rotate-half is
                # folded into two shifted multiplies with a negated sin
                u = prp.tile([128, 4, DH], F32, tag="u", name="u")
                nc.vector.tensor_mul(out=u, in0=pp, in1=bc_heads(cosT[:, t, :]))
                w = prp.tile([128, 4, 3, 32], F32, tag="wv", name="wv")
                nc.vector.tensor_mul(out=w[:, :, :, 0:16],
                                     in0=pp[:, :, :, 16:32],
                                     in1=bc_heads(half16(nsinT, t, 0)))
                nc.vector.tensor_mul(out=w[:, :, :, 16:32],
                                     in0=pp[:, :, :, 0:16],
                                     in1=bc_heads(half16(sinT, t, 1)))
                qk = prp.tile([128, 4, DH], IN_DT, tag="qk", name="qk")
                nc.vector.tensor_add(out=qk, in0=u, in1=w)
                dest = QTs if c < 2 else KTs
                h0 = (c % 2) * 4
                for hh in range(4):
                    tp = pstp.tile([DH, 128], IN_DT, tag="tp", name="tp")
                    nc.tensor.transpose(out=tp, in_=qk[:, hh, :],
                                        identity=ident_sb)
                    dst = dest[h0 + hh][:, t * 128:(t + 1) * 128]
                    if hh % 2 == 0:
                        nc.vector.tensor_copy(out=dst, in_=tp)
                    else:
                        nc.scalar.copy(out=dst, in_=tp)

    # ---- attention + output projection ----------------------------------
    NG = 2          # key chunks per score group (one exp instruction each)
    NGRP = NMT // NG
    with (
        tc.tile_pool(name="patt", bufs=1) as pat,
        tc.tile_pool(name="pex", bufs=2) as pex,
        tc.tile_pool(name="pdt", bufs=2) as pdt,
        tc.tile_pool(name="posb", bufs=2) as posb,
        tc.tile_pool(name="psc", bufs=2, space="PSUM") as psc,
        tc.tile_pool(name="psho", bufs=2, space="PSUM") as psho,
        tc.tile_pool(name="pse", bufs=2, space="PSUM") as pse,
    ):
        WoSB = pat.tile([DH, HC, DIM], IN_DT, tag="wo")
        nc.sync.dma_start(out=WoSB, in_=WoC.rearrange("(h p) c -> p h c", p=DH))
        bias_sb = pat.tile([128, DIM], F32, tag="bias")
        bout_bc = bass.AP(tensor=boutC.tensor, offset=boutC.offset,
                          ap=[[0, 128]] + [list(p) for p in boutC.ap])
        nc.sync.dma_start(out=bias_sb, in_=bout_bc)
        hoU = pat.tile([DH, 2, HC, TT], IN_DT, tag="hoU")
        hoT = pat.tile([DH, HC, TT], IN_DT, tag="hoT")
        dn = pat.tile([HC, 2, TT], F32, tag="dn")
        dnf = pat.tile([HC, 2, TT], F32R, tag="dnf")
        rcp = pat.tile([1, 2, HC, TT], F32R, tag="rcp")

        # finish work for the previous query tile is emitted piecewise
        # between score/AV groups so the PE fills exp-wait bubbles. Pacing
        # spreads the pieces across all of the next tile's pop slots.
        filler = []
        pace = [0]

        def pop_filler(force=False):
            if not filler:
                return
            pace[0] += len(filler)
            if force or pace[0] >= 64:
                pace[0] -= 64
                filler.pop(0)()

        def attn_unit(qt, h):
            """scores -> exp -> AV for (query tile qt, head h).

            Group-level software pipeline: sc(g) runs while exp(g-1) is on
            the scalar engine; av(g-1) follows sc(g)."""
            par = qt % 2
            qsl = slice(qt * TT, (qt + 1) * TT)
            ho_ps = psho.tile([DH + 1, TT], F32, tag="hops", name="hops")
            ex = [None, None]

            def sc_group(g):
                scp = psc.tile([128, NG, TT], F32, tag="scps", name="scps")
                for j in range(NG):
                    p = NG * g + j
                    nc.tensor.matmul(
                        out=scp[:, j, :],
                        lhsT=KTs[h][:, p * 128:(p + 1) * 128],
                        rhs=QTs[h][:, qsl], start=True, stop=True,
                    )
                ex[g % 2] = pex.tile([128, NG, TT], IN_DT, tag="ex", name="ex")
                nc.scalar.activation(out=ex[g % 2], in_=scp, func=AF.Exp,
                                     scale=SCALE)

            def av_group(g):
                for j in range(NG):
                    p = NG * g + j
                    nc.tensor.matmul(
                        out=ho_ps, lhsT=Vt[:, p, h, :], rhs=ex[g % 2][:, j, :],
                        start=(p == 0), stop=(p == NMT - 1),
                    )

            sc_group(0)
            for g in range(1, NGRP):
                sc_group(g)
                av_group(g - 1)
                pop_filler()
            av_group(NGRP - 1)
            # stash unnormalized numerator + denominator row (the
            # denominator goes via same-partition copy + DMA shuffle);
            # scalar handles the last tile so vector is free for the tail
            dt = pdt.tile([DH + 1, TT], F32, tag="dt", name="dt")
            if qt == NQT - 1:
                nc.scalar.copy(out=hoU[:, par, h, :], in_=ho_ps[0:DH, :])
                nc.scalar.copy(out=dt[DH:DH + 1, :], in_=ho_ps[DH:DH + 1, :])
            else:
                nc.vector.tensor_copy(out=hoU[:, par, h, :],
                                      in_=ho_ps[0:DH, :])
                nc.vector.tensor_copy(out=dt[DH:DH + 1, :],
                                      in_=ho_ps[DH:DH + 1, :])
            nc.sync.dma_start(out=dn[h:h + 1, par, :], in_=dt[DH:DH + 1, :])

        def fin_recip(qt):
            par = qt % 2
            with nc.allow_low_precision(reason="f32r is bitwise f32"):
                nc.vector.reciprocal(out=dnf[:, par, :], in_=dn[:, par, :])
            for h in range(HC):
                nc.sync.dma_start(out=rcp[0:1, par, h, :],
                                  in_=dnf[h:h + 1, par, :])

        def fin_norm(qt, h):
            par = qt % 2
            bc = psho.tile([DH + 1, TT], F32, tag="hops", name="bc")
            nc.tensor.matmul(out=bc[0:DH, :], lhsT=ones1,
                             rhs=rcp[0:1, par, h, :], start=True, stop=True)
            nc.vector.tensor_mul(out=hoT[:, h, :], in0=hoU[:, par, h, :],
                                 in1=bc[0:DH, :])

        def fin_chain(qt, tt_i, et):
            row = qt * TT + tt_i * 128
            eps = pse.tile([128, TT], F32, tag="eps", name="eps")
            for h in range(HC):
                nc.tensor.matmul(
                    out=eps,
                    lhsT=hoT[:, h, tt_i * 128:(tt_i + 1) * 128],
                    rhs=WoSB[:, h, et * TT:(et + 1) * TT],
                    start=(h == 0), stop=(h == HC - 1),
                )
            osb = posb.tile([128, TT], F32, tag="osb", name="osb")
            nc.vector.tensor_add(
                out=osb, in0=eps, in1=bias_sb[:, et * TT:(et + 1) * TT]
            )
            nc.sync.dma_start(
                out=out[row:row + 128, et * TT:(et + 1) * TT], in_=osb,
            )

        def queue_finish(qt):
            filler.append(lambda qt=qt: fin_recip(qt))
            for h in range(HC):
                filler.append(lambda qt=qt, h=h: fin_norm(qt, h))
            for tt_i in range(TT // 128):
                for et in range(DIM // TT):
                    filler.append(
                        lambda qt=qt, t=tt_i, e=et: fin_chain(qt, t, e))

        for qt in range(NQT):
            for h in range(HC):
                attn_unit(qt, h)
                pop_filler()
            queue_finish(qt)
        while filler:
            pop_filler(force=True)


def build():
    from contextlib import ExitStack

    nc = bacc.Bacc("TRN2", target_bir_lowering=False, debug=False)
    xT_p = nc.declare_dram_parameter("xT", [DIM, N], IN_DT, isOutput=False)
    fN_p = nc.declare_dram_parameter("fN", [N, DH], F32, isOutput=False)
    id_p = nc.declare_dram_parameter("ident", [128, 128], IN_DT, isOutput=False)
    Wq_p = nc.declare_dram_parameter("Wq", [DIM, HD], IN_DT, isOutput=False)
    Wk_p = nc.declare_dram_parameter("Wk", [DIM, HD], IN_DT, isOutput=False)
    Wv_p = nc.declare_dram_parameter("Wv", [DIM, HD], IN_DT, isOutput=False)
    WoC_p = nc.declare_dram_parameter("WoC", [HD, DIM], IN_DT, isOutput=False)
    bout_p = nc.declare_dram_parameter("boutC", [DIM], F32, isOutput=False)
    out = nc.declare_dram_parameter("out", [N, DIM], F32, isOutput=True)
    io = tuple(
        t[:] for t in (xT_p, fN_p, id_p, Wq_p, Wk_p, Wv_p, WoC_p, bout_p, out)
    )
    with ExitStack() as ctx:
        tc = ctx.enter_context(tile.TileContext(nc))
        _emit(ctx, tc, io)
    nc.finalize()
    return nc


def make_in_maps(x, f1, f2, f3, Wqkv, Wout, bout):
    x = np.asarray(x, np.float32)
    fcat = np.concatenate(
        [np.asarray(f1, np.float32), np.asarray(f2, np.float32),
         np.asarray(f3, np.float32)], axis=1,
    )  # [N, DH]
    fN_np = np.ascontiguousarray(fcat)
    ident_np = np.eye(128, dtype=np.float32).astype(IN_NP)
    Wqkv = np.asarray(Wqkv, np.float32)
    Wout = np.asarray(Wout, np.float32)
    bout = np.ascontiguousarray(np.asarray(bout, np.float32))
    zbias = np.zeros_like(bout)
    xTs = [np.ascontiguousarray(x[b].T).astype(IN_NP) for b in range(B)]
    Wslice = []
    for hh in range(2):
        cs = hh * HD
        Wslice.append(dict(
            Wq=np.ascontiguousarray(Wqkv[:, cs:cs + HD]).astype(IN_NP),
            Wk=np.ascontiguousarray(Wqkv[:, H * DH + cs:H * DH + cs + HD]).astype(IN_NP),
            Wv=np.ascontiguousarray(Wqkv[:, 2 * H * DH + cs:2 * H * DH + cs + HD]).astype(IN_NP),
            WoC=np.ascontiguousarray(Wout[cs:cs + HD, :]).astype(IN_NP),
        ))
    in_maps = []
    for c in range(8):
        b, hh = divmod(c, 2)
        in_maps.append(dict(
            xT=xTs[b], fN=fN_np, ident=ident_np,
            boutC=bout if hh == 0 else zbias,
            **Wslice[hh],
        ))
    return in_maps


_NC_CACHE = None


def kernel(x, f1, f2, f3, Wqkv, Wout, bout, _trace=False):
    global _NC_CACHE
    if _NC_CACHE is None:
        _NC_CACHE = build()
    nc = _NC_CACHE
    in_maps = make_in_maps(x, f1, f2, f3, Wqkv, Wout, bout)
    res = run_bass_kernel_spmd(nc, in_maps, list(range(8)), trace=_trace)
    out = np.empty((B, N, DIM), np.float32)
    for b in range(B):
        np.add(res.results[2 * b]["out"], res.results[2 * b + 1]["out"],
               out=out[b])
    if _trace:
        return out, res
    return out
